# revision 1
# baseline (speedup 1.0000x reference)
"""MinGRU Trainium2 kernel (v3).

Reference computation (per batch b):
    c = depthwise_conv1d(x, conv_w, taps=5, pad=2)        # [D, L]
    h = h_w @ c                                           # [O, L]
    g = concat([-1000, +1000], g_w @ c)                   # [O, L]
    a = sigmoid(-g); v = (1 - a) * h
    out[l] = a[l] * out[l-1] + v[l]     (linear scan along L)

Strategy: pure data-parallel over B (8 batches -> 8 NeuronCores).
Per core, stream in l-chunks of 512:
  - conv: 5 diagonal-matmuls per d-tile on TensorE accumulating in PSUM,
    x and the diagonal conv weights in bf16 (conv error ~0.5% vs the
    2e-2 gate).
  - c PSUM->SBUF copies on ScalarE (fp32r); h/g 1x1-conv matmuls fp32r.
  - a = sigmoid(-(g + bias)) on ScalarE (bias carries +/-1000 polarized
    rows 0/1, built on-chip from a partition iota); z = 1 - a on GpSimd;
    v = z*h on VectorE; scan via tensor_tensor_scan (DVE).
  - rows 0/1 come out of the scan naturally (a saturates to exactly 1/0):
    row0 = 0, row1 = h, within tolerance of the reference's log-domain
    quantization on channel 1 -> no end-pass.
  - x is relaid out host-side to [128, chunk, dt, 516] WITH the +/-2
    halos baked in (edges zero-padded), so each chunk loads with ONE
    fully-contiguous DMA (chunk 0 split in two so conv starts sooner).
  - out is written as [128, (ot, chunk, col)] and permuted host-side so
    each chunk stores with ONE DMA; chunk 7 runs its sigmoid/z/v/scan
    chain on 256-col halves with per-o-tile stores to shorten the tail.
"""

import numpy as np
import ml_dtypes

import concourse.bass as bass
import concourse.mybir as mybir
from concourse import bacc
from concourse.tile import TileContext
from concourse.bass_utils import run_bass_kernel_spmd

F32 = mybir.dt.float32
F32R = mybir.dt.float32r
BF16 = mybir.dt.bfloat16
I32 = mybir.dt.int32
AF = mybir.ActivationFunctionType
OP = mybir.AluOpType

B, D, O, L = 8, 512, 512, 4096
P = 128
CH = 512                 # l-chunk width (one PSUM bank)
NCH = L // CH            # 8
NDT = D // P             # 4 d-tiles
NOT = O // P             # 4 o-tiles
NTAPS = 5
N_CORES = 8
XW = CH + 4              # x tile width incl halos


def build_program():
    nc = bacc.Bacc()

    xrh = nc.declare_dram_parameter("xrh", [P, NCH * NDT * XW], BF16,
                                    isOutput=False)
    cwd = nc.declare_dram_parameter("cwd", [P, NDT * NTAPS * P], BF16,
                                    isOutput=False)
    hwTr = nc.declare_dram_parameter("hwTr", [P, NOT * NDT * P], BF16,
                                     isOutput=False)
    gwTr = nc.declare_dram_parameter("gwTr", [P, NOT * NDT * P], BF16,
                                     isOutput=False)
    outr = nc.declare_dram_parameter("outr", [P, NOT * L], F32, isOutput=True)

    xr4 = xrh.rearrange("p (i dt l) -> p i dt l", dt=NDT, l=XW)
    outr3 = outr.rearrange("p (ot l) -> p ot l", l=L)

    with TileContext(nc) as tc:
        with (
            tc.tile_pool(name="weights", bufs=1) as wpool,
            tc.tile_pool(name="xin", bufs=3) as xpool,
            tc.tile_pool(name="csb", bufs=3) as cpool,
            tc.tile_pool(name="actout", bufs=6) as apool,
            tc.tile_pool(name="vtiles", bufs=4) as vpool,
            tc.tile_pool(name="outt", bufs=3) as opool,
            tc.tile_pool(name="cps", bufs=3, space="PSUM") as cps_pool,
            tc.tile_pool(name="hps", bufs=3, space="PSUM") as hps_pool,
            tc.tile_pool(name="gps", bufs=2, space="PSUM") as gps_pool,
        ):
            # ---- PE warm-up: memset a tiny tile (no DMA wait) and run dummy
            # matmuls to trip the HAM clock gate to 2.4 GHz early and keep it
            # busy until the first x/cw tiles land.
            warm_sb = wpool.tile([P, 2], BF16, tag="warm")
            nc.gpsimd.memset(warm_sb, 0.0)
            wps = cps_pool.tile([P, CH], F32, tag="cps", name="warmps")
            for _ in range(48):
                nc.tensor.matmul(wps[0:2, 0:2], lhsT=warm_sb, rhs=warm_sb,
                                 start=True, stop=True)
            wout = wpool.tile([2, 2], F32, tag="warmout")
            nc.vector.tensor_copy(wout, wps[0:2, 0:2])

            # ---- weight DMAs on the Scalar HWDGE queue (x owns Sync).
            # conv diagonals first (conv(0) needs them), per-d-tile; then h/g
            # weights in o-tile halves so rest(0) doesn't wait for the full
            # load.
            cw_sb = wpool.tile([P, NDT * NTAPS, P], BF16, tag="cw")
            CB = NTAPS * P
            for dt in range(NDT):
                nc.scalar.dma_start(
                    out=cw_sb[:, dt * NTAPS:(dt + 1) * NTAPS, :],
                    in_=cwd[:, dt * CB:(dt + 1) * CB])
            gwTr_sb = wpool.tile([P, NOT * NDT * P], BF16, tag="gwTr")
            hwTr_sb = wpool.tile([P, NOT * NDT * P], BF16, tag="hwTr")
            HB = 2 * NDT * P  # columns per o-tile-pair block (1024)
            nc.scalar.dma_start(out=gwTr_sb[:, 0:HB], in_=gwTr[:, 0:HB])
            nc.scalar.dma_start(out=hwTr_sb[:, 0:HB], in_=hwTr[:, 0:HB])
            nc.scalar.dma_start(out=gwTr_sb[:, HB:2 * HB],
                                in_=gwTr[:, HB:2 * HB])
            nc.scalar.dma_start(out=hwTr_sb[:, HB:2 * HB],
                                in_=hwTr[:, HB:2 * HB])

            # ---- polarizing bias for o-tile 0 (rows 0/1 of g):
            # gbn0[p] = 1000*(p==0) - 1000*(p==1), built from a partition iota
            pidx = wpool.tile([P, 1], I32, tag="pidx")
            nc.gpsimd.iota(pidx, pattern=[[1, 1]], base=0, channel_multiplier=1)
            gbn0 = wpool.tile([P, 1], F32, tag="gbn0")
            gbn1 = wpool.tile([P, 1], F32, tag="gbn1")
            nc.gpsimd.tensor_scalar(gbn0, pidx, 0, 1000.0, OP.is_equal, OP.mult)
            nc.gpsimd.tensor_scalar(gbn1, pidx, 1, -1000.0, OP.is_equal, OP.mult)
            nc.gpsimd.tensor_tensor(gbn0, gbn0, gbn1, OP.add)
            gbn0n = wpool.tile([P, 1], F32, tag="gbn0n")
            nc.gpsimd.tensor_scalar(gbn0n, gbn0, -1.0, None, OP.mult)

            c_sb = [None] * NCH       # [chunk] -> [128, 4, 512] f32r tile
            prev_out = [None]         # previous chunk's big out tile

            def emit_conv(i):
                xt = xpool.tile([P, NDT, XW], BF16, tag="xt")
                if i == 0:
                    # split so the d-tile-0 conv starts after 1/4 the bytes
                    nc.sync.dma_start(out=xt[:, 0:1, :], in_=xr4[:, 0, 0:1, :])
                    nc.sync.dma_start(out=xt[:, 1:2, :], in_=xr4[:, 0, 1:2, :])
                    nc.sync.dma_start(out=xt[:, 2:4, :], in_=xr4[:, 0, 2:4, :])
                else:
                    nc.sync.dma_start(out=xt, in_=xr4[:, i, :, :])
                ct = cpool.tile([P, NDT, CH], BF16, tag="ct")
                tap_order = (2, 0, 1, 3, 4)
                for dt in range(NDT):
                    cp = cps_pool.tile([P, CH], F32, tag="cps")
                    for j, k in enumerate(tap_order):
                        nc.tensor.matmul(
                            cp,
                            lhsT=cw_sb[:, dt * NTAPS + k, :],
                            rhs=xt[:, dt, k:k + CH],
                            start=(j == 0), stop=(j == NTAPS - 1),
                        )
                    nc.scalar.copy(ct[:, dt, :], cp)
                c_sb[i] = ct

            def emit_rest(i):
                ott = opool.tile([P, NOT, CH], F32, tag="outt")
                last = i == NCH - 1
                halves = 2 if last else 1
                HW_ = CH // halves

                def emit_g(ot):
                    gp = gps_pool.tile([P, CH], F32, tag="gps")
                    for dt in range(NDT):
                        nc.tensor.matmul(
                            gp,
                            lhsT=gwTr_sb[:, ot * 512 + dt * P:
                                         ot * 512 + dt * P + P],
                            rhs=c_sb[i][:, dt, :],
                            start=(dt == 0), stop=(dt == NDT - 1),
                        )
                    at = apool.tile([P, CH], F32, tag="at")
                    zt = vpool.tile([P, CH], F32, tag="zt")
                    for hf in range(halves):
                        sl = slice(hf * HW_, (hf + 1) * HW_)
                        nc.scalar.activation(at[:, sl], gp[:, sl], AF.Sigmoid,
                                             bias=(gbn0[:, :] if ot == 0
                                                   else 0.0),
                                             scale=-1.0)
                        nc.gpsimd.tensor_scalar(zt[:, sl], at[:, sl],
                                                -1.0, 1.0, OP.mult, OP.add)
                    return at, zt

                def emit_h(ot, at, zt):
                    hp = hps_pool.tile([P, CH], F32, tag="hps")
                    for dt in range(NDT):
                        nc.tensor.matmul(
                            hp,
                            lhsT=hwTr_sb[:, ot * 512 + dt * P:
                                         ot * 512 + dt * P + P],
                            rhs=c_sb[i][:, dt, :],
                            start=(dt == 0), stop=(dt == NDT - 1),
                        )
                    vt = vpool.tile([P, CH], F32, tag="vt")
                    for hf in range(halves):
                        sl = slice(hf * HW_, (hf + 1) * HW_)
                        nc.vector.tensor_tensor(vt[:, sl], zt[:, sl],
                                                hp[:, sl], OP.mult)
                        if hf == 0:
                            init = (0.0 if i == 0
                                    else prev_out[0][:, ot, CH - 1:CH])
                        else:
                            init = ott[:, ot, hf * HW_ - 1:hf * HW_]
                        nc.vector.tensor_tensor_scan(
                            ott[:, ot, sl], at[:, sl], vt[:, sl], init,
                            OP.mult, OP.add)
                        if last:
                            # Sync is idle at the drain; keep Scalar's queue
                            # free for the sigmoid/z chain
                            nc.sync.dma_start(
                                out=outr3[:, ot, i * CH + hf * HW_:
                                          i * CH + (hf + 1) * HW_],
                                in_=ott[:, ot, sl])

                if last:
                    # all g matmuls first: every o-tile's sigmoid/z finishes
                    # while the h matmuls still stream, shortening the drain
                    az = [emit_g(ot) for ot in range(NOT)]
                    for ot in range(NOT):
                        emit_h(ot, *az[ot])
                else:
                    for ot in range(NOT):
                        at, zt = emit_g(ot)
                        emit_h(ot, at, zt)
                if not last:
                    nc.scalar.dma_start(
                        out=outr3[:, :, i * CH:(i + 1) * CH], in_=ott)
                prev_out[0] = ott

            # chunk pairs, software-pipelined one pair ahead
            emit_conv(0)
            emit_conv(1)
            for p in range(1, NCH // 2):
                emit_conv(2 * p)
                emit_conv(2 * p + 1)
                emit_rest(2 * p - 2)
                emit_rest(2 * p - 1)
            emit_rest(NCH - 2)
            emit_rest(NCH - 1)

    nc.finalize()
    return nc


_PROGRAM = None


def _get_program():
    global _PROGRAM
    if _PROGRAM is None:
        _PROGRAM = build_program()
    return _PROGRAM


def prepare_in_maps(x, conv_w, h_w, g_w):
    x = np.ascontiguousarray(np.asarray(x), dtype=np.float32)
    conv_w = np.asarray(conv_w, dtype=np.float32)
    h_w = np.asarray(h_w, dtype=np.float32)
    g_w = np.asarray(g_w, dtype=np.float32)

    # hwTr[p, ot*512 + dt*128 + m] = h_w[ot*128+m, dt*128+p]  (bf16)
    hw = h_w[:, :, 0]                                             # [O, D]
    hwTr = np.ascontiguousarray(
        hw.reshape(NOT, P, NDT, P).transpose(3, 0, 2, 1).reshape(P, -1)
        .astype(ml_dtypes.bfloat16))

    gwp = np.zeros((O, D), np.float32)
    gwp[2:, :] = g_w[:, :, 0]
    gwTr = np.ascontiguousarray(
        gwp.reshape(NOT, P, NDT, P).transpose(3, 0, 2, 1).reshape(P, -1)
        .astype(ml_dtypes.bfloat16))

    # cwd[p, (dt*5+k)*128 + q] = (q == p) * conv_w[dt*128+p, 0, k]  (bf16)
    cwd = np.zeros((P, NDT * NTAPS, P), np.float32)
    q = np.arange(P)
    for dt in range(NDT):
        for k in range(NTAPS):
            cwd[q, dt * NTAPS + k, q] = conv_w[dt * P + q, 0, k]
    cwd = np.ascontiguousarray(cwd.reshape(P, -1).astype(ml_dtypes.bfloat16))

    # xrh[p, i, dt, j] = xpad[dt*128+p, i*512 + j] with 2-col zero halos
    in_maps = []
    for b in range(B):
        xpad = np.pad(x[b], ((0, 0), (2, 2))).astype(ml_dtypes.bfloat16)
        xpad = xpad.reshape(NDT, P, L + 4)
        xb = np.empty((P, NCH, NDT, XW), dtype=ml_dtypes.bfloat16)
        for i in range(NCH):
            xb[:, i, :, :] = xpad[:, :, i * CH:i * CH + XW].transpose(1, 0, 2)
        in_maps.append({"xrh": np.ascontiguousarray(xb.reshape(P, -1)),
                        "cwd": cwd, "hwTr": hwTr, "gwTr": gwTr})
    return in_maps


def kernel(x, conv_w, h_w, g_w):
    in_maps = prepare_in_maps(x, conv_w, h_w, g_w)
    nc = _get_program()
    res = run_bass_kernel_spmd(nc, in_maps, list(range(N_CORES))).results
    return np.stack(
        [res[b]["outr"].reshape(P, NOT, L).transpose(1, 0, 2).reshape(O, L)
         for b in range(B)], axis=0)

